# revision 7
# baseline (speedup 1.0000x reference)
"""Trainium2 Bass kernel for nn_GCNModel (GCN with instance-norm encoder).

Self-contained: hardcodes all shapes. Shards 50000 nodes across 8 NeuronCores
(6250 each). Host prep: transpose x to feature-major bf16, build edge chunks
(dst-sorted, 128-dst blocks), selector tiles, gather indices. Device: stats
pass -> AllReduce -> fold inorm into weights -> encoder MLPs -> GCN convs via
AllGather + indirect gather + selector matmuls -> head -> sigmoid.
"""
import numpy as np
import ml_dtypes

bf16 = ml_dtypes.bfloat16

R = 8                 # cores
N = 50000             # nodes
NS = N // R           # 6250 nodes per core
NSP = 6272            # padded shard rows (49*128) for hw tables
F1 = 3072             # branch-1 input feats
F2 = 227              # branch-2 input feats
FIN = 3328            # padded xT rows (26*128): 3072 + 256
H1 = 1536
H3 = 3072
GC = 128              # GCN width
EPS = 1e-5

NCHUNKS = [512] * 12 + [106]          # node chunks per core
NOFF = [512 * i for i in range(13)]
NBLK = 49                              # 128-dst blocks per core (last=106)

_CACHE = {}


# ----------------------------------------------------------------------------
# host-side prep
# ----------------------------------------------------------------------------

def _prep_edges(edge_index):
    src = np.asarray(edge_index[0], dtype=np.int64)
    dst = np.asarray(edge_index[1], dtype=np.int64)
    deg = (np.bincount(dst, minlength=N).astype(np.float32) + 1.0)
    dinv = (1.0 / np.sqrt(deg)).astype(np.float32)

    asrc = np.concatenate([src, np.arange(N, dtype=np.int64)])
    adst = np.concatenate([dst, np.arange(N, dtype=np.int64)])
    acoef = np.concatenate([dinv[src] * dinv[dst], dinv * dinv]).astype(np.float32)

    rank = adst // NS
    dloc = adst - rank * NS
    blk = dloc >> 7
    rb = rank * NBLK + blk

    cnt = np.bincount(rb, minlength=R * NBLK).reshape(R, NBLK)
    C = np.maximum((cnt + 127) // 128, 1).max(axis=0)      # chunks per block
    offs = np.concatenate([[0], np.cumsum(C)]).astype(np.int64)
    TOTC = int(offs[-1])

    order = np.argsort(rb, kind="stable")
    srb = rb[order]
    grp_start = np.searchsorted(srb, np.arange(R * NBLK))
    pos = np.arange(len(order)) - grp_start[srb]

    srcrow = ((asrc // NS) * NSP + (asrc % NS)).astype(np.int32)

    SELs, IDXs, NEGSs = [], [], []
    for r in range(R):
        m = (rank[order] == r)
        er = order[m]
        b = blk[er]
        p = pos[m]
        c = offs[b] + (p >> 7)
        j = p & 127
        dl = (dloc[er] - b * 128).astype(np.int64)
        sel = np.zeros((TOTC, 128, 128), np.float32)
        sel[c, j, dl] = acoef[er]
        SELs.append(sel.astype(bf16))
        idxt = np.zeros((128, TOTC), np.int32)
        idxt[j, c] = srcrow[er]
        IDXs.append(idxt)
        S = np.bincount(dloc[er], weights=acoef[er].astype(np.float64), minlength=NS)
        negs = np.zeros((1, NSP), np.float32)
        negs[0, :NS] = -S.astype(np.float32)
        NEGSs.append(negs.astype(bf16))
    return tuple(C.tolist()), TOTC, SELs, IDXs, NEGSs


def _pad(a, shape):
    out = np.zeros(shape, a.dtype)
    out[tuple(slice(0, s) for s in a.shape)] = a
    return out


def _prep_weights(inp):
    f32 = np.float32
    w = {}
    w["w_e1"] = np.asarray(inp["w_e1"], f32).astype(bf16)          # [3072,1536]
    w["w_e2"] = np.asarray(inp["w_e2"], f32).astype(bf16)          # [1536,1536]
    w["w_e3"] = np.asarray(inp["w_e3"], f32).astype(bf16)          # [1536,3072]
    w["w_e4"] = _pad(np.asarray(inp["w_e4"], f32), (256, 128)).astype(bf16)
    w["w_e5"] = _pad(np.asarray(inp["w_e5"], f32), (128, 128)).astype(bf16)
    w["w_e6"] = _pad(np.asarray(inp["w_e6"], f32), (128, 256)).astype(bf16)
    wc1 = np.asarray(inp["w_c1"], f32)                             # [3299,128]
    w["wc1a"] = wc1[:F1].astype(bf16)                              # [3072,128]
    w["wc1b"] = _pad(wc1[F1:], (256, 128)).astype(bf16)            # [256,128]
    w["w_c2"] = np.asarray(inp["w_c2"], f32).astype(bf16)          # [128,128]
    w["w_f1"] = np.asarray(inp["w_f1"], f32).astype(bf16)          # [128,64]
    w["w_f2"] = np.asarray(inp["w_f2"], f32).astype(bf16)          # [64,1]
    # biases in [128, nchunks] feature-major layout
    w["b_e1"] = np.asarray(inp["b_e1"], f32).reshape(12, 128).T.copy()
    w["b_e2"] = np.asarray(inp["b_e2"], f32).reshape(12, 128).T.copy()
    w["b_e3"] = np.asarray(inp["b_e3"], f32).reshape(24, 128).T.copy()
    w["b_e4"] = _pad(np.asarray(inp["b_e4"], f32)[:, None], (128, 1))
    w["b_e5"] = _pad(np.asarray(inp["b_e5"], f32)[:, None], (128, 1))
    w["b_e6"] = _pad(np.asarray(inp["b_e6"], f32), (256,)).reshape(2, 128).T.copy()
    w["b_c1"] = np.asarray(inp["b_c1"], f32)[:, None].copy()       # [128,1]
    w["b_c2"] = np.asarray(inp["b_c2"], f32)[:, None].copy()
    w["b_f1"] = _pad(np.asarray(inp["b_f1"], f32)[:, None], (64, 1))
    w["b_f2"] = np.asarray(inp["b_f2"], f32).reshape(1, 1).copy()
    w["ident"] = np.eye(128, dtype=f32)
    return w


# ----------------------------------------------------------------------------
# device kernel build
# ----------------------------------------------------------------------------

def _build(C, TOTC):
    from concourse import bass, bacc, mybir, tile

    dt = mybir.dt
    AF = mybir.ActivationFunctionType
    ALU = mybir.AluOpType
    AX = mybir.AxisListType

    nc = bacc.Bacc("TRN2", target_bir_lowering=False, debug=False,
                   enable_asserts=False, num_devices=R)

    # ---- I/O ----
    xT = nc.dram_tensor("xT", [FIN, NS], dt.bfloat16, kind="ExternalInput")
    W = {}
    for nm, sh in [("w_e1", [F1, H1]), ("w_e2", [H1, H1]), ("w_e3", [H1, H3]),
                   ("w_e4", [256, 128]), ("w_e5", [128, 128]), ("w_e6", [128, 256]),
                   ("wc1a", [F1, GC]), ("wc1b", [256, GC]), ("w_c2", [GC, GC]),
                   ("w_f1", [GC, 64]), ("w_f2", [64, 1])]:
        W[nm] = nc.dram_tensor(nm, sh, dt.bfloat16, kind="ExternalInput")
    B = {}
    for nm, sh in [("b_e1", [128, 12]), ("b_e2", [128, 12]), ("b_e3", [128, 24]),
                   ("b_e4", [128, 1]), ("b_e5", [128, 1]), ("b_e6", [128, 2]),
                   ("b_c1", [128, 1]), ("b_c2", [128, 1]), ("b_f1", [64, 1]),
                   ("b_f2", [1, 1]), ("ident", [128, 128])]:
        B[nm] = nc.dram_tensor(nm, sh, dt.float32, kind="ExternalInput")
    SEL = nc.dram_tensor("SEL", [TOTC, 128, 128], dt.bfloat16, kind="ExternalInput")
    IDXT = nc.dram_tensor("IDXT", [128, TOTC], dt.int32, kind="ExternalInput")
    NEGS = nc.dram_tensor("NEGS", [1, NSP], dt.bfloat16, kind="ExternalInput")
    out_d = nc.dram_tensor("out", [1, NS], dt.float32, kind="ExternalOutput")

    # ---- internal DRAM ----
    h1T = nc.dram_tensor("h1T", [H1, NS], dt.bfloat16, kind="Internal")
    h2T = nc.dram_tensor("h2T", [H1, NS], dt.bfloat16, kind="Internal")
    h3T = nc.dram_tensor("h3T", [H3, NS], dt.bfloat16, kind="Internal")
    h4T = nc.dram_tensor("h4T", [128, NS], dt.bfloat16, kind="Internal")
    h5T = nc.dram_tensor("h5T", [128, NS], dt.bfloat16, kind="Internal")
    h6T = nc.dram_tensor("h6T", [256, NS], dt.bfloat16, kind="Internal")
    hw_sh = [nc.dram_tensor(f"hw{i}_sh", [NSP, GC], dt.bfloat16, kind="Internal")
             for i in (1, 2)]
    hw_full = [nc.dram_tensor(f"hw{i}_full", [NSP * R, GC], dt.bfloat16,
                              kind="Internal", addr_space="Shared") for i in (1, 2)]
    st_io = []
    for i, cch in enumerate([26, 13, 13, 26]):
        si = nc.dram_tensor(f"st{i}_in", [128, cch, 2], dt.float32, kind="Internal")
        so = nc.dram_tensor(f"st{i}_out", [128, cch, 2], dt.float32,
                            kind="Internal", addr_space="Shared")
        st_io.append((si, so))

    rg = [list(range(R))]

    with tile.TileContext(nc) as tc:
        with tc.tile_pool(name="persist", bufs=1) as PP, \
             tc.tile_pool(name="wpool", bufs=1) as WP, \
             tc.tile_pool(name="io", bufs=2) as IO, \
             tc.tile_pool(name="io3", bufs=3) as IO3, \
             tc.tile_pool(name="psA", bufs=3, space="PSUM") as PS, \
             tc.tile_pool(name="psV", bufs=2, space="PSUM") as PSV, \
             tc.tile_pool(name="sc", bufs=2) as SC:

            # persistent small tiles
            bias_sb = {}
            for nm in B:
                bias_sb[nm] = PP.tile(list(B[nm].shape), dt.float32, name=f"sb_{nm}")
                nc.sync.dma_start(out=bias_sb[nm][:], in_=B[nm][:])
            idxt_sb = PP.tile([128, TOTC], dt.int32)
            nc.sync.dma_start(out=idxt_sb[:], in_=IDXT[:])
            hg1 = PP.tile([128, NSP], dt.bfloat16)   # conv1 output, feature-major
            hg2 = PP.tile([128, NSP], dt.bfloat16)
            epsb = PP.tile([128, 1], dt.float32)
            nc.vector.memset(epsb[:], EPS)

            def allreduce_stats(stat, pair):
                si, so = pair
                nc.sync.dma_start(out=si[:], in_=stat[:])
                nc.gpsimd.collective_compute(
                    "AllReduce", ALU.add, replica_groups=rg,
                    ins=[si[:]], outs=[so[:]])
                nc.sync.dma_start(out=stat[:], in_=so[:])

            def mu_s_t(stat, cch, nm):
                """AR'd stat -> (mu_bf16, s_f32, t=s*mu bf16) [128, cch] tiles."""
                mu = PP.tile([128, cch], dt.float32, name=f"mu_{nm}")
                mub = PP.tile([128, cch], dt.bfloat16, name=f"mub_{nm}")
                s_ = PP.tile([128, cch], dt.float32, name=f"s_{nm}")
                nc.vector.tensor_scalar_mul(out=mu[:], in0=stat[:, :, 0],
                                            scalar1=1.0 / N)
                e2 = SC.tile([128, cch], dt.float32, tag="e2")
                nc.vector.tensor_scalar_mul(out=e2[:], in0=stat[:, :, 1],
                                            scalar1=1.0 / N)
                m2 = SC.tile([128, cch], dt.float32, tag="m2")
                nc.vector.tensor_tensor(out=m2[:], in0=mu[:], in1=mu[:], op=ALU.mult)
                nc.vector.tensor_tensor(out=e2[:], in0=e2[:], in1=m2[:],
                                        op=ALU.subtract)
                nc.scalar.activation(out=e2[:], in_=e2[:], func=AF.Sqrt,
                                     bias=epsb[:, 0:1])
                nc.vector.reciprocal(out=s_[:], in_=e2[:])
                nc.vector.tensor_copy(out=mub[:], in_=mu[:])
                return mub, s_

            def fold_weight(wdram, KC, MC, s_ap, k0, tag="wp"):
                """W' = s (rows) * W -> bf16 tile [128, KC, MC, 128]."""
                wp = WP.tile([128, KC, MC, 128], dt.bfloat16, tag=tag,
                             name=f"{tag}_{wdram.name}")
                for k in range(KC):
                    raw = SC.tile([128, MC * 128], dt.bfloat16, tag="wraw", bufs=1)
                    nc.sync.dma_start(out=raw[:], in_=wdram[k * 128:(k + 1) * 128, :])
                    nc.vector.tensor_scalar(
                        out=wp[:, k], in0=raw[:].rearrange("p (m q) -> p m q", q=128),
                        scalar1=s_ap[:, k0 + k:k0 + k + 1], scalar2=None,
                        op0=ALU.mult)
                return wp

            def bias_fold(wp, KC, MC, mub, k0, braw, nm):
                """b' = b - mu @ W' -> [128, MC] f32 (uses folded weights)."""
                bp = PP.tile([128, MC], dt.float32, name=f"bp_{nm}")
                for m in range(MC):
                    pv = PSV.tile([128, 1], dt.float32, tag="pv")
                    for k in range(KC):
                        nc.tensor.matmul(out=pv[:], lhsT=wp[:, k, m],
                                         rhs=mub[:, k0 + k:k0 + k + 1],
                                         start=(k == 0), stop=(k == KC - 1))
                    nc.vector.tensor_tensor(out=bp[:, m:m + 1], in0=braw[:, m:m + 1],
                                            in1=pv[:], op=ALU.subtract)
                return bp

            def enc_layer(src_ap, dst_ap, KC, MC, wp, bp, stat, c0, sfx=""):
                """dst = relu(W'.T @ src + b'), feature-major, stats accumulated."""
                for n in range(13):
                    w_ = NCHUNKS[n]
                    xin = IO.tile([128, KC, 512], dt.bfloat16, tag="xin" + sfx)
                    nc.sync.dma_start(
                        out=xin[:, :, :w_],
                        in_=src_ap[:, NOFF[n]:NOFF[n] + w_].rearrange(
                            "(c p) n -> p c n", p=128))
                    for m in range(MC):
                        ps = PS.tile([128, 512], dt.float32, tag="mm")
                        for k in range(KC):
                            nc.tensor.matmul(out=ps[:, :w_], lhsT=wp[:, k, m],
                                             rhs=xin[:, k, :w_],
                                             start=(k == 0), stop=(k == KC - 1))
                        hout = IO3.tile([128, 512], dt.bfloat16, tag="hout" + sfx)
                        nc.scalar.activation(out=hout[:, :w_], in_=ps[:, :w_],
                                             func=AF.Relu, bias=bp[:, m:m + 1])
                        nc.sync.dma_start(
                            out=dst_ap[m * 128:(m + 1) * 128,
                                       NOFF[n]:NOFF[n] + w_],
                            in_=hout[:, :w_])
                        if stat is not None:
                            part = SC.tile([128, 1], dt.float32, tag="part")
                            nc.vector.tensor_reduce(out=part[:], in_=hout[:, :w_],
                                                    axis=AX.X, op=ALU.add)
                            nc.vector.tensor_tensor(
                                out=stat[:, c0 + m, 0:1], in0=stat[:, c0 + m, 0:1],
                                in1=part[:], op=ALU.add)
                            sq = SC.tile([128, 512], dt.bfloat16, tag="sq")
                            nc.scalar.activation(out=sq[:, :w_], in_=hout[:, :w_],
                                                 func=AF.Square)
                            nc.vector.tensor_reduce(out=part[:], in_=sq[:, :w_],
                                                    axis=AX.X, op=ALU.add)
                            nc.vector.tensor_tensor(
                                out=stat[:, c0 + m, 1:2], in0=stat[:, c0 + m, 1:2],
                                in1=part[:], op=ALU.add)

            # ================= phase 0: x stats =================
            xstat = PP.tile([128, 26, 2], dt.float32)
            nc.vector.memset(xstat[:], 0.0)
            for n in range(13):
                w_ = NCHUNKS[n]
                xin = IO.tile([128, 26, 512], dt.bfloat16, tag="xin")
                nc.sync.dma_start(
                    out=xin[:, :, :w_],
                    in_=xT[:, NOFF[n]:NOFF[n] + w_].rearrange(
                        "(c p) n -> p c n", p=128))
                part = SC.tile([128, 26], dt.float32, tag="part26")
                nc.vector.tensor_reduce(out=part[:], in_=xin[:, :, :w_],
                                        axis=AX.X, op=ALU.add)
                nc.vector.tensor_tensor(out=xstat[:, :, 0], in0=xstat[:, :, 0],
                                        in1=part[:], op=ALU.add)
                for c in range(26):
                    sq = SC.tile([128, 512], dt.bfloat16, tag="sq")
                    nc.scalar.activation(out=sq[:, :w_], in_=xin[:, c, :w_],
                                         func=AF.Square)
                    p1 = SC.tile([128, 1], dt.float32, tag="part")
                    nc.vector.tensor_reduce(out=p1[:], in_=sq[:, :w_],
                                            axis=AX.X, op=ALU.add)
                    nc.vector.tensor_tensor(out=xstat[:, c, 1:2],
                                            in0=xstat[:, c, 1:2],
                                            in1=p1[:], op=ALU.add)
            allreduce_stats(xstat, st_io[0])
            mub_x, s_x = mu_s_t(xstat, 26, "x")

            # ================= e1 + e4 =================
            st1 = PP.tile([128, 13, 2], dt.float32)
            nc.vector.memset(st1[:], 0.0)
            wp1 = fold_weight(W["w_e1"], 24, 12, s_x, 0)
            bp1 = bias_fold(wp1, 24, 12, mub_x, 0, bias_sb["b_e1"], "e1")
            enc_layer(xT[:F1, :], h1T[:], 24, 12, wp1, bp1, st1, 0)
            wp4 = fold_weight(W["w_e4"], 2, 1, s_x, 24, tag="wpb")
            bp4 = bias_fold(wp4, 2, 1, mub_x, 24, bias_sb["b_e4"], "e4")
            enc_layer(xT[F1:FIN, :], h4T[:], 2, 1, wp4, bp4, st1, 12, sfx="b")
            allreduce_stats(st1, st_io[1])
            mub1, s1 = mu_s_t(st1, 13, "1")

            # ================= e2 + e5 =================
            st2 = PP.tile([128, 13, 2], dt.float32)
            nc.vector.memset(st2[:], 0.0)
            wp2 = fold_weight(W["w_e2"], 12, 12, s1, 0)
            bp2 = bias_fold(wp2, 12, 12, mub1, 0, bias_sb["b_e2"], "e2")
            enc_layer(h1T[:], h2T[:], 12, 12, wp2, bp2, st2, 0)
            wp5 = fold_weight(W["w_e5"], 1, 1, s1, 12, tag="wpb")
            bp5 = bias_fold(wp5, 1, 1, mub1, 12, bias_sb["b_e5"], "e5")
            enc_layer(h4T[:], h5T[:], 1, 1, wp5, bp5, st2, 12, sfx="b")
            allreduce_stats(st2, st_io[2])
            mub2, s2 = mu_s_t(st2, 13, "2")

            # ================= e3 + e6 =================
            st3 = PP.tile([128, 26, 2], dt.float32)
            nc.vector.memset(st3[:], 0.0)
            wp3 = fold_weight(W["w_e3"], 12, 24, s2, 0)
            bp3 = bias_fold(wp3, 12, 24, mub2, 0, bias_sb["b_e3"], "e3")
            enc_layer(h2T[:], h3T[:], 12, 24, wp3, bp3, st3, 0)
            wp6 = fold_weight(W["w_e6"], 1, 2, s2, 12, tag="wpb")
            bp6 = bias_fold(wp6, 1, 2, mub2, 12, bias_sb["b_e6"], "e6")
            enc_layer(h5T[:], h6T[:], 1, 2, wp6, bp6, st3, 24, sfx="b")
            allreduce_stats(st3, st_io[3])
            mub3, s3 = mu_s_t(st3, 26, "3")

            # ================= fold wc1 + c0 =================
            wpc = WP.tile([128, 26, 128], dt.bfloat16, tag="wpc")
            for k in range(26):
                wdram = W["wc1a"] if k < 24 else W["wc1b"]
                kk = k if k < 24 else k - 24
                raw = SC.tile([128, 128], dt.bfloat16, tag="wvt")
                nc.sync.dma_start(out=raw[:], in_=wdram[kk * 128:(kk + 1) * 128, :])
                nc.vector.tensor_scalar(out=wpc[:, k], in0=raw[:],
                                        scalar1=s3[:, k:k + 1], scalar2=None,
                                        op0=ALU.mult)
            pv = PSV.tile([128, 1], dt.float32, tag="pv")
            for k in range(26):
                nc.tensor.matmul(out=pv[:], lhsT=wpc[:, k],
                                 rhs=mub3[:, k:k + 1],
                                 start=(k == 0), stop=(k == 25))
            c0f = PP.tile([128, 1], dt.float32)
            nc.vector.tensor_copy(out=c0f[:], in_=pv[:])
            pt = PSV.tile([1, 128], dt.float32, tag="pv")
            nc.tensor.transpose(out=pt[:], in_=c0f[:], identity=bias_sb["ident"][:])
            c0m = PP.tile([1, 128], dt.bfloat16)
            nc.vector.tensor_copy(out=c0m[:], in_=pt[:])

            # ================= hw1 = hhat @ wc1 (node-major) =================
            def hw1_table():
                for nb in range(NBLK):
                    w_ = 128 if nb < NBLK - 1 else 106
                    n0 = nb * 128
                    ps = PS.tile([128, 128], dt.float32, tag="small")
                    ta = IO.tile([128, 24, 128], dt.bfloat16, tag="hin")
                    nc.sync.dma_start(
                        out=ta[:, :, :w_],
                        in_=h3T[:, n0:n0 + w_].rearrange("(c p) n -> p c n", p=128))
                    tb_ = IO.tile([128, 2, 128], dt.bfloat16, tag="hinb")
                    nc.sync.dma_start(
                        out=tb_[:, :, :w_],
                        in_=h6T[:, n0:n0 + w_].rearrange("(c p) n -> p c n", p=128))
                    for k in range(26):
                        lt = ta[:, k, :w_] if k < 24 else tb_[:, k - 24, :w_]
                        nc.tensor.matmul(out=ps[:w_, :], lhsT=lt, rhs=wpc[:, k],
                                         start=(k == 0), stop=(k == 25))
                    hw_sb = IO3.tile([128, 128], dt.bfloat16, tag="hwsb")
                    nc.vector.tensor_copy(out=hw_sb[:w_, :], in_=ps[:w_, :])
                    nc.sync.dma_start(out=hw_sh[0][n0:n0 + w_, :], in_=hw_sb[:w_, :])

            hw1_table()
            nc.gpsimd.collective_compute(
                "AllGather", ALU.bypass, replica_groups=rg,
                ins=[hw_sh[0][:]], outs=[hw_full[0][:]])

            def aggregate(hwf, hgdst, bias_ap, with_c0):
                ci = 0
                for b in range(NBLK):
                    ps = PS.tile([128, 128], dt.float32, tag="small")
                    nsteps = C[b] + (1 if with_c0 else 0)
                    step = 0
                    if with_c0:
                        negt = IO3.tile([1, 128], dt.bfloat16, tag="negt")
                        nc.sync.dma_start(out=negt[:],
                                          in_=NEGS[:, b * 128:(b + 1) * 128])
                        nc.tensor.matmul(out=ps[:], lhsT=c0m[:], rhs=negt[:],
                                         start=True, stop=(nsteps == 1))
                        step += 1
                    for c in range(C[b]):
                        g_t = IO3.tile([128, 128], dt.bfloat16, tag="g")
                        nc.gpsimd.indirect_dma_start(
                            out=g_t[:], out_offset=None, in_=hwf[:],
                            in_offset=bass.IndirectOffsetOnAxis(
                                ap=idxt_sb[:, ci:ci + 1], axis=0))
                        sel_t = IO3.tile([128, 128], dt.bfloat16, tag="sel")
                        nc.sync.dma_start(out=sel_t[:], in_=SEL[ci])
                        nc.tensor.matmul(out=ps[:], lhsT=g_t[:], rhs=sel_t[:],
                                         start=(step == 0), stop=(step == nsteps - 1))
                        step += 1
                        ci += 1
                    nc.scalar.activation(out=hgdst[:, b * 128:(b + 1) * 128],
                                         in_=ps[:], func=AF.Relu, bias=bias_ap)

            aggregate(hw_full[0], hg1, bias_sb["b_c1"][:, 0:1], True)

            # ================= conv2 =================
            wc2_sb = PP.tile([128, 128], dt.bfloat16)
            nc.sync.dma_start(out=wc2_sb[:], in_=W["w_c2"][:])
            for nb in range(NBLK):
                w_ = 128 if nb < NBLK - 1 else 106
                n0 = nb * 128
                ps = PS.tile([128, 128], dt.float32, tag="small")
                nc.tensor.matmul(out=ps[:w_, :], lhsT=hg1[:, n0:n0 + w_],
                                 rhs=wc2_sb[:], start=True, stop=True)
                hw_sb = IO3.tile([128, 128], dt.bfloat16, tag="hwsb")
                nc.vector.tensor_copy(out=hw_sb[:w_, :], in_=ps[:w_, :])
                nc.sync.dma_start(out=hw_sh[1][n0:n0 + w_, :], in_=hw_sb[:w_, :])
            nc.gpsimd.collective_compute(
                "AllGather", ALU.bypass, replica_groups=rg,
                ins=[hw_sh[1][:]], outs=[hw_full[1][:]])
            aggregate(hw_full[1], hg2, bias_sb["b_c2"][:, 0:1], False)

            # ================= head =================
            wf1_sb = PP.tile([128, 64], dt.bfloat16)
            nc.sync.dma_start(out=wf1_sb[:], in_=W["w_f1"][:])
            wf2_sb = PP.tile([64, 1], dt.bfloat16)
            nc.sync.dma_start(out=wf2_sb[:], in_=W["w_f2"][:])
            for n in range(13):
                w_ = NCHUNKS[n]
                p1 = PS.tile([64, 512], dt.float32, tag="mm")
                nc.tensor.matmul(out=p1[:, :w_], lhsT=wf1_sb[:],
                                 rhs=hg2[:, NOFF[n]:NOFF[n] + w_],
                                 start=True, stop=True)
                f1sb = IO3.tile([64, 512], dt.bfloat16, tag="f1sb")
                nc.scalar.activation(out=f1sb[:, :w_], in_=p1[:, :w_],
                                     func=AF.Relu, bias=bias_sb["b_f1"][:, 0:1])
                p2 = PS.tile([1, 512], dt.float32, tag="mm")
                nc.tensor.matmul(out=p2[:, :w_], lhsT=wf2_sb[:], rhs=f1sb[:, :w_],
                                 start=True, stop=True)
                osb = IO3.tile([1, 512], dt.float32, tag="osb")
                nc.scalar.activation(out=osb[:, :w_], in_=p2[:, :w_],
                                     func=AF.Sigmoid, bias=bias_sb["b_f2"][:, 0:1])
                nc.sync.dma_start(out=out_d[:, NOFF[n]:NOFF[n] + w_],
                                  in_=osb[:, :w_])

    nc.compile()
    return nc


# ----------------------------------------------------------------------------
# entry point
# ----------------------------------------------------------------------------

def kernel(**inputs):
    import jax  # noqa: F401  (triggers axon boot via sitecustomize)
    from concourse import bass_utils

    x = np.asarray(inputs["x"], np.float32)
    edge_index = np.asarray(inputs["edge_index"])

    eb = edge_index.tobytes()
    key = ("k", edge_index.shape, hash(eb[:4096]), hash(eb[-4096:]))
    if key not in _CACHE:
        C, TOTC, SELs, IDXs, NEGSs = _prep_edges(edge_index)
        nc = _build(C, TOTC)
        _CACHE.clear()
        _CACHE[key] = (nc, C, TOTC, SELs, IDXs, NEGSs)
    nc, C, TOTC, SELs, IDXs, NEGSs = _CACHE[key]

    w = _prep_weights(inputs)
    in_maps = []
    for r in range(R):
        xr = np.zeros((FIN, NS), bf16)
        xr[:3299] = np.ascontiguousarray(x[r * NS:(r + 1) * NS].T).astype(bf16)
        m = {"xT": xr, "SEL": SELs[r], "IDXT": IDXs[r], "NEGS": NEGSs[r]}
        for nm in ["w_e1", "w_e2", "w_e3", "w_e4", "w_e5", "w_e6",
                   "wc1a", "wc1b", "w_c2", "w_f1", "w_f2",
                   "b_e1", "b_e2", "b_e3", "b_e4", "b_e5", "b_e6",
                   "b_c1", "b_c2", "b_f1", "b_f2", "ident"]:
            m[nm] = w[nm]
        in_maps.append(m)

    res = bass_utils.run_bass_kernel_spmd(nc, in_maps, core_ids=list(range(R)))
    out = np.concatenate([np.asarray(res.results[r]["out"][0], np.float32)
                          for r in range(R)])
    return out[:, None].astype(np.float32)


# revision 11
# speedup vs baseline: 1.0365x; 1.0365x over previous
"""Trainium2 Bass kernel for nn_GCNModel (GCN with instance-norm encoder).

Self-contained: hardcodes all shapes. Shards 50000 nodes across 8 NeuronCores
(6250 each). Host prep: transpose x to feature-major bf16, build edge chunks
(dst-sorted, 128-dst blocks), selector tiles, gather indices. Device: stats
pass -> AllReduce -> fold inorm into weights -> encoder MLPs -> GCN convs via
AllGather + indirect gather + selector matmuls -> head -> sigmoid.
"""
import numpy as np
import ml_dtypes

bf16 = ml_dtypes.bfloat16

R = 8                 # cores
N = 50000             # nodes
NS = N // R           # 6250 nodes per core
NSP = 6272            # padded shard rows (49*128) for hw tables
F1 = 3072             # branch-1 input feats
F2 = 227              # branch-2 input feats
FIN = 3328            # padded xT rows (26*128): 3072 + 256
H1 = 1536
H3 = 3072
GC = 128              # GCN width
EPS = 1e-5

NCHUNKS = [512] * 12 + [106]          # node chunks per core
NOFF = [512 * i for i in range(13)]
NBLK = 49                              # 128-dst blocks per core (last=106)

_CACHE = {}


# ----------------------------------------------------------------------------
# host-side prep
# ----------------------------------------------------------------------------

def _prep_edges(edge_index):
    src = np.asarray(edge_index[0], dtype=np.int64)
    dst = np.asarray(edge_index[1], dtype=np.int64)
    deg = (np.bincount(dst, minlength=N).astype(np.float32) + 1.0)
    dinv = (1.0 / np.sqrt(deg)).astype(np.float32)

    asrc = np.concatenate([src, np.arange(N, dtype=np.int64)])
    adst = np.concatenate([dst, np.arange(N, dtype=np.int64)])
    acoef = np.concatenate([dinv[src] * dinv[dst], dinv * dinv]).astype(np.float32)

    rank = adst // NS
    dloc = adst - rank * NS
    blk = dloc >> 7
    rb = rank * NBLK + blk

    cnt = np.bincount(rb, minlength=R * NBLK).reshape(R, NBLK)
    C = np.maximum((cnt + 127) // 128, 1).max(axis=0)      # chunks per block
    offs = np.concatenate([[0], np.cumsum(C)]).astype(np.int64)
    TOTC = int(offs[-1])

    order = np.argsort(rb, kind="stable")
    srb = rb[order]
    grp_start = np.searchsorted(srb, np.arange(R * NBLK))
    pos = np.arange(len(order)) - grp_start[srb]

    srcrow = ((asrc // NS) * NSP + (asrc % NS)).astype(np.int32)

    SELs, IDXs, NEGSs = [], [], []
    for r in range(R):
        m = (rank[order] == r)
        er = order[m]
        b = blk[er]
        p = pos[m]
        c = offs[b] + (p >> 7)
        j = p & 127
        dl = (dloc[er] - b * 128).astype(np.int64)
        sel = np.zeros((TOTC, 128, 128), np.float32)
        sel[c, j, dl] = acoef[er]
        SELs.append(sel.astype(bf16))
        idxt = np.zeros((128, TOTC), np.int32)
        idxt[j, c] = srcrow[er]
        IDXs.append(idxt)
        S = np.bincount(dloc[er], weights=acoef[er].astype(np.float64), minlength=NS)
        negs = np.zeros((1, NSP), np.float32)
        negs[0, :NS] = -S.astype(np.float32)
        NEGSs.append(negs.astype(bf16))
    return tuple(C.tolist()), TOTC, SELs, IDXs, NEGSs


def _pad(a, shape):
    out = np.zeros(shape, a.dtype)
    out[tuple(slice(0, s) for s in a.shape)] = a
    return out


def _prep_weights(inp):
    f32 = np.float32
    w = {}
    w["w_e1"] = np.asarray(inp["w_e1"], f32).astype(bf16)          # [3072,1536]
    w["w_e2"] = np.asarray(inp["w_e2"], f32).astype(bf16)          # [1536,1536]
    w["w_e3"] = np.asarray(inp["w_e3"], f32).astype(bf16)          # [1536,3072]
    w["w_e4"] = _pad(np.asarray(inp["w_e4"], f32), (256, 128)).astype(bf16)
    w["w_e5"] = _pad(np.asarray(inp["w_e5"], f32), (128, 128)).astype(bf16)
    w["w_e6"] = _pad(np.asarray(inp["w_e6"], f32), (128, 256)).astype(bf16)
    wc1 = np.asarray(inp["w_c1"], f32)                             # [3299,128]
    w["wc1a"] = wc1[:F1].astype(bf16)                              # [3072,128]
    w["wc1b"] = _pad(wc1[F1:], (256, 128)).astype(bf16)            # [256,128]
    w["w_c2"] = np.asarray(inp["w_c2"], f32).astype(bf16)          # [128,128]
    w["w_f1"] = np.asarray(inp["w_f1"], f32).astype(bf16)          # [128,64]
    w["w_f2"] = np.asarray(inp["w_f2"], f32).astype(bf16)          # [64,1]
    # biases in [128, nchunks] feature-major layout
    w["b_e1"] = np.asarray(inp["b_e1"], f32).reshape(12, 128).T.copy()
    w["b_e2"] = np.asarray(inp["b_e2"], f32).reshape(12, 128).T.copy()
    w["b_e3"] = np.asarray(inp["b_e3"], f32).reshape(24, 128).T.copy()
    w["b_e4"] = _pad(np.asarray(inp["b_e4"], f32)[:, None], (128, 1))
    w["b_e5"] = _pad(np.asarray(inp["b_e5"], f32)[:, None], (128, 1))
    w["b_e6"] = _pad(np.asarray(inp["b_e6"], f32), (256,)).reshape(2, 128).T.copy()
    w["b_c1"] = np.asarray(inp["b_c1"], f32)[:, None].copy()       # [128,1]
    w["b_c2"] = np.asarray(inp["b_c2"], f32)[:, None].copy()
    w["b_f1"] = _pad(np.asarray(inp["b_f1"], f32)[:, None], (64, 1))
    w["b_f2"] = np.asarray(inp["b_f2"], f32).reshape(1, 1).copy()
    w["ident"] = np.eye(128, dtype=f32)
    return w


# ----------------------------------------------------------------------------
# device kernel build
# ----------------------------------------------------------------------------

def _build(C, TOTC):
    from concourse import bass, bacc, mybir, tile

    dt = mybir.dt
    AF = mybir.ActivationFunctionType
    ALU = mybir.AluOpType
    AX = mybir.AxisListType

    nc = bacc.Bacc("TRN2", target_bir_lowering=False, debug=False,
                   enable_asserts=False, num_devices=R)

    # ---- I/O ----
    xT = nc.dram_tensor("xT", [FIN, NS], dt.bfloat16, kind="ExternalInput")
    W = {}
    for nm, sh in [("w_e1", [F1, H1]), ("w_e2", [H1, H1]), ("w_e3", [H1, H3]),
                   ("w_e4", [256, 128]), ("w_e5", [128, 128]), ("w_e6", [128, 256]),
                   ("wc1a", [F1, GC]), ("wc1b", [256, GC]), ("w_c2", [GC, GC]),
                   ("w_f1", [GC, 64]), ("w_f2", [64, 1])]:
        W[nm] = nc.dram_tensor(nm, sh, dt.bfloat16, kind="ExternalInput")
    B = {}
    for nm, sh in [("b_e1", [128, 12]), ("b_e2", [128, 12]), ("b_e3", [128, 24]),
                   ("b_e4", [128, 1]), ("b_e5", [128, 1]), ("b_e6", [128, 2]),
                   ("b_c1", [128, 1]), ("b_c2", [128, 1]), ("b_f1", [64, 1]),
                   ("b_f2", [1, 1]), ("ident", [128, 128])]:
        B[nm] = nc.dram_tensor(nm, sh, dt.float32, kind="ExternalInput")
    SEL = nc.dram_tensor("SEL", [TOTC, 128, 128], dt.bfloat16, kind="ExternalInput")
    IDXT = nc.dram_tensor("IDXT", [128, TOTC], dt.int32, kind="ExternalInput")
    NEGS = nc.dram_tensor("NEGS", [1, NSP], dt.bfloat16, kind="ExternalInput")
    out_d = nc.dram_tensor("out", [1, NS], dt.float32, kind="ExternalOutput")

    # ---- internal DRAM ----
    h1T = nc.dram_tensor("h1T", [H1, NS], dt.bfloat16, kind="Internal")
    h2T = nc.dram_tensor("h2T", [H1, NS], dt.bfloat16, kind="Internal")
    h3T = nc.dram_tensor("h3T", [H3, NS], dt.bfloat16, kind="Internal")
    h4T = nc.dram_tensor("h4T", [128, NS], dt.bfloat16, kind="Internal")
    h5T = nc.dram_tensor("h5T", [128, NS], dt.bfloat16, kind="Internal")
    h6T = nc.dram_tensor("h6T", [256, NS], dt.bfloat16, kind="Internal")
    hw_sh = [nc.dram_tensor(f"hw{i}_sh", [NSP, GC], dt.bfloat16, kind="Internal")
             for i in (1, 2)]
    hw_full = [nc.dram_tensor(f"hw{i}_full", [NSP * R, GC], dt.bfloat16,
                              kind="Internal", addr_space="Shared") for i in (1, 2)]
    st_io = []
    for i, cch in enumerate([26, 13, 13, 26]):
        si = nc.dram_tensor(f"st{i}_in", [128, cch, 2], dt.float32, kind="Internal")
        so = nc.dram_tensor(f"st{i}_out", [128, cch, 2], dt.float32,
                            kind="Internal", addr_space="Shared")
        st_io.append((si, so))

    rg = [list(range(R))]

    with tile.TileContext(nc) as tc:
        with tc.tile_pool(name="persist", bufs=1) as PP, \
             tc.tile_pool(name="wpool", bufs=1) as WP, \
             tc.tile_pool(name="io", bufs=2) as IO, \
             tc.tile_pool(name="io3", bufs=3) as IO3, \
             tc.tile_pool(name="psA", bufs=3, space="PSUM") as PS, \
             tc.tile_pool(name="psV", bufs=2, space="PSUM") as PSV, \
             tc.tile_pool(name="sc", bufs=2) as SC:

            # persistent small tiles
            bias_sb = {}
            for nm in B:
                bias_sb[nm] = PP.tile(list(B[nm].shape), dt.float32, name=f"sb_{nm}")
                nc.sync.dma_start(out=bias_sb[nm][:], in_=B[nm][:])
            idxt_sb = PP.tile([128, TOTC], dt.int32)
            nc.sync.dma_start(out=idxt_sb[:], in_=IDXT[:])
            hg1 = PP.tile([128, NSP], dt.bfloat16)   # conv1 output, feature-major
            hg2 = PP.tile([128, NSP], dt.bfloat16)
            epsb = PP.tile([128, 1], dt.float32)
            nc.vector.memset(epsb[:], EPS)

            def allreduce_stats(stat, pair):
                si, so = pair
                nc.sync.dma_start(out=si[:], in_=stat[:])
                nc.gpsimd.collective_compute(
                    "AllReduce", ALU.add, replica_groups=rg,
                    ins=[si[:]], outs=[so[:]])
                nc.sync.dma_start(out=stat[:], in_=so[:])

            def mu_s_t(stat, cch, nm):
                """AR'd stat -> (mu_bf16, s_f32, t=s*mu bf16) [128, cch] tiles."""
                mu = PP.tile([128, cch], dt.float32, name=f"mu_{nm}")
                mub = PP.tile([128, cch], dt.bfloat16, name=f"mub_{nm}")
                s_ = PP.tile([128, cch], dt.float32, name=f"s_{nm}")
                nc.vector.tensor_scalar_mul(out=mu[:], in0=stat[:, :, 0],
                                            scalar1=1.0 / N)
                e2 = SC.tile([128, cch], dt.float32, tag="e2")
                nc.vector.tensor_scalar_mul(out=e2[:], in0=stat[:, :, 1],
                                            scalar1=1.0 / N)
                m2 = SC.tile([128, cch], dt.float32, tag="m2")
                nc.vector.tensor_tensor(out=m2[:], in0=mu[:], in1=mu[:], op=ALU.mult)
                nc.vector.tensor_tensor(out=e2[:], in0=e2[:], in1=m2[:],
                                        op=ALU.subtract)
                nc.scalar.activation(out=e2[:], in_=e2[:], func=AF.Sqrt,
                                     bias=epsb[:, 0:1])
                nc.vector.reciprocal(out=s_[:], in_=e2[:])
                nc.vector.tensor_copy(out=mub[:], in_=mu[:])
                return mub, s_

            def fold_weight(wdram, KC, MC, s_ap, k0, tag="wp"):
                """W' = s (rows) * W -> bf16 tile [128, KC, MC, 128]."""
                wp = WP.tile([128, KC, MC, 128], dt.bfloat16, tag=tag,
                             name=f"{tag}_{wdram.name}")
                for k in range(KC):
                    raw = SC.tile([128, MC * 128], dt.bfloat16, tag="wraw", bufs=1)
                    nc.sync.dma_start(out=raw[:], in_=wdram[k * 128:(k + 1) * 128, :])
                    nc.vector.tensor_scalar(
                        out=wp[:, k], in0=raw[:].rearrange("p (m q) -> p m q", q=128),
                        scalar1=s_ap[:, k0 + k:k0 + k + 1], scalar2=None,
                        op0=ALU.mult)
                return wp

            def bias_fold(wp, KC, MC, mub, k0, braw, nm):
                """b' = b - mu @ W' -> [128, MC] f32 (uses folded weights)."""
                bp = PP.tile([128, MC], dt.float32, name=f"bp_{nm}")
                for m in range(MC):
                    pv = PSV.tile([128, 1], dt.float32, tag="pv")
                    for k in range(KC):
                        nc.tensor.matmul(out=pv[:], lhsT=wp[:, k, m],
                                         rhs=mub[:, k0 + k:k0 + k + 1],
                                         start=(k == 0), stop=(k == KC - 1))
                    nc.vector.tensor_tensor(out=bp[:, m:m + 1], in0=braw[:, m:m + 1],
                                            in1=pv[:], op=ALU.subtract)
                return bp

            def enc_layer(src_ap, dst_ap, KC, MC, wp, bp, stat, c0, sfx=""):
                """dst = relu(W'.T @ src + b'), feature-major, stats accumulated."""
                for n in range(13):
                    w_ = NCHUNKS[n]
                    xin = IO.tile([128, KC, 512], dt.bfloat16, tag="xin" + sfx)
                    nc.sync.dma_start(
                        out=xin[:, :, :w_],
                        in_=src_ap[:, NOFF[n]:NOFF[n] + w_].rearrange(
                            "(c p) n -> p c n", p=128))
                    if stat is not None:
                        psum_p = SC.tile([128, MC], dt.float32, tag="pp" + sfx)
                        psq_p = SC.tile([128, MC], dt.float32, tag="pq" + sfx)
                    for m in range(MC):
                        ps = PS.tile([128, 512], dt.float32, tag="mm")
                        for k in range(KC):
                            nc.tensor.matmul(out=ps[:, :w_], lhsT=wp[:, k, m],
                                             rhs=xin[:, k, :w_],
                                             start=(k == 0), stop=(k == KC - 1))
                        hout = IO3.tile([128, 512], dt.bfloat16, tag="hout" + sfx)
                        nc.scalar.activation(out=hout[:, :w_], in_=ps[:, :w_],
                                             func=AF.Relu, bias=bp[:, m:m + 1])
                        nc.sync.dma_start(
                            out=dst_ap[m * 128:(m + 1) * 128,
                                       NOFF[n]:NOFF[n] + w_],
                            in_=hout[:, :w_])
                        if stat is not None:
                            nc.vector.tensor_reduce(out=psum_p[:, m:m + 1],
                                                    in_=hout[:, :w_],
                                                    axis=AX.X, op=ALU.add)
                            sq = SC.tile([128, 512], dt.bfloat16, tag="sq")
                            nc.scalar.activation(out=sq[:, :w_], in_=hout[:, :w_],
                                                 func=AF.Square,
                                                 accum_out=psq_p[:, m:m + 1])
                    if stat is not None:
                        nc.vector.tensor_tensor(
                            out=stat[:, c0:c0 + MC, 0], in0=stat[:, c0:c0 + MC, 0],
                            in1=psum_p[:], op=ALU.add)
                        nc.vector.tensor_tensor(
                            out=stat[:, c0:c0 + MC, 1], in0=stat[:, c0:c0 + MC, 1],
                            in1=psq_p[:], op=ALU.add)

            # ================= phase 0: x stats =================
            xstat = PP.tile([128, 26, 2], dt.float32)
            nc.vector.memset(xstat[:], 0.0)
            for n in range(13):
                w_ = NCHUNKS[n]
                xin = IO.tile([128, 26, 512], dt.bfloat16, tag="xin")
                nc.sync.dma_start(
                    out=xin[:, :, :w_],
                    in_=xT[:, NOFF[n]:NOFF[n] + w_].rearrange(
                        "(c p) n -> p c n", p=128))
                part = SC.tile([128, 26], dt.float32, tag="part26")
                nc.vector.tensor_reduce(out=part[:], in_=xin[:, :, :w_],
                                        axis=AX.X, op=ALU.add)
                nc.vector.tensor_tensor(out=xstat[:, :, 0], in0=xstat[:, :, 0],
                                        in1=part[:], op=ALU.add)
                psq = SC.tile([128, 26], dt.float32, tag="psq26")
                for c in range(26):
                    sq = SC.tile([128, 512], dt.bfloat16, tag="sq")
                    nc.scalar.activation(out=sq[:, :w_], in_=xin[:, c, :w_],
                                         func=AF.Square,
                                         accum_out=psq[:, c:c + 1])
                nc.vector.tensor_tensor(out=xstat[:, :, 1], in0=xstat[:, :, 1],
                                        in1=psq[:], op=ALU.add)
            allreduce_stats(xstat, st_io[0])
            mub_x, s_x = mu_s_t(xstat, 26, "x")

            # ================= e1 + e4 =================
            st1 = PP.tile([128, 13, 2], dt.float32)
            nc.vector.memset(st1[:], 0.0)
            wp1 = fold_weight(W["w_e1"], 24, 12, s_x, 0)
            bp1 = bias_fold(wp1, 24, 12, mub_x, 0, bias_sb["b_e1"], "e1")
            enc_layer(xT[:F1, :], h1T[:], 24, 12, wp1, bp1, st1, 0)
            wp4 = fold_weight(W["w_e4"], 2, 1, s_x, 24, tag="wpb")
            bp4 = bias_fold(wp4, 2, 1, mub_x, 24, bias_sb["b_e4"], "e4")
            enc_layer(xT[F1:FIN, :], h4T[:], 2, 1, wp4, bp4, st1, 12, sfx="b")
            allreduce_stats(st1, st_io[1])
            mub1, s1 = mu_s_t(st1, 13, "1")

            # ================= e2 + e5 =================
            st2 = PP.tile([128, 13, 2], dt.float32)
            nc.vector.memset(st2[:], 0.0)
            wp2 = fold_weight(W["w_e2"], 12, 12, s1, 0)
            bp2 = bias_fold(wp2, 12, 12, mub1, 0, bias_sb["b_e2"], "e2")
            enc_layer(h1T[:], h2T[:], 12, 12, wp2, bp2, st2, 0)
            wp5 = fold_weight(W["w_e5"], 1, 1, s1, 12, tag="wpb")
            bp5 = bias_fold(wp5, 1, 1, mub1, 12, bias_sb["b_e5"], "e5")
            enc_layer(h4T[:], h5T[:], 1, 1, wp5, bp5, st2, 12, sfx="b")
            allreduce_stats(st2, st_io[2])
            mub2, s2 = mu_s_t(st2, 13, "2")

            # ================= e3 + e6 =================
            st3 = PP.tile([128, 26, 2], dt.float32)
            nc.vector.memset(st3[:], 0.0)
            wp3 = fold_weight(W["w_e3"], 12, 24, s2, 0)
            bp3 = bias_fold(wp3, 12, 24, mub2, 0, bias_sb["b_e3"], "e3")
            enc_layer(h2T[:], h3T[:], 12, 24, wp3, bp3, st3, 0)
            wp6 = fold_weight(W["w_e6"], 1, 2, s2, 12, tag="wpb")
            bp6 = bias_fold(wp6, 1, 2, mub2, 12, bias_sb["b_e6"], "e6")
            enc_layer(h5T[:], h6T[:], 1, 2, wp6, bp6, st3, 24, sfx="b")
            allreduce_stats(st3, st_io[3])
            mub3, s3 = mu_s_t(st3, 26, "3")

            # ================= fold wc1 + c0 =================
            wpc = WP.tile([128, 26, 128], dt.bfloat16, tag="wpc")
            for k in range(26):
                wdram = W["wc1a"] if k < 24 else W["wc1b"]
                kk = k if k < 24 else k - 24
                raw = SC.tile([128, 128], dt.bfloat16, tag="wvt")
                nc.sync.dma_start(out=raw[:], in_=wdram[kk * 128:(kk + 1) * 128, :])
                nc.vector.tensor_scalar(out=wpc[:, k], in0=raw[:],
                                        scalar1=s3[:, k:k + 1], scalar2=None,
                                        op0=ALU.mult)
            pv = PSV.tile([128, 1], dt.float32, tag="pv")
            for k in range(26):
                nc.tensor.matmul(out=pv[:], lhsT=wpc[:, k],
                                 rhs=mub3[:, k:k + 1],
                                 start=(k == 0), stop=(k == 25))
            c0f = PP.tile([128, 1], dt.float32)
            nc.vector.tensor_copy(out=c0f[:], in_=pv[:])
            pt = PSV.tile([1, 128], dt.float32, tag="pv")
            nc.tensor.transpose(out=pt[:], in_=c0f[:], identity=bias_sb["ident"][:])
            c0m = PP.tile([1, 128], dt.bfloat16)
            nc.vector.tensor_copy(out=c0m[:], in_=pt[:])

            # ================= hw1 = hhat @ wc1 (node-major) =================
            def hw1_table():
                for nb in range(NBLK):
                    w_ = 128 if nb < NBLK - 1 else 106
                    n0 = nb * 128
                    ps = PS.tile([128, 128], dt.float32, tag="small")
                    ta = IO.tile([128, 24, 128], dt.bfloat16, tag="xin")
                    nc.sync.dma_start(
                        out=ta[:, :, :w_],
                        in_=h3T[:, n0:n0 + w_].rearrange("(c p) n -> p c n", p=128))
                    tb_ = IO.tile([128, 2, 128], dt.bfloat16, tag="xinb")
                    nc.sync.dma_start(
                        out=tb_[:, :, :w_],
                        in_=h6T[:, n0:n0 + w_].rearrange("(c p) n -> p c n", p=128))
                    for k in range(26):
                        lt = ta[:, k, :w_] if k < 24 else tb_[:, k - 24, :w_]
                        nc.tensor.matmul(out=ps[:w_, :], lhsT=lt, rhs=wpc[:, k],
                                         start=(k == 0), stop=(k == 25))
                    hw_sb = IO3.tile([128, 128], dt.bfloat16, tag="hwsb")
                    nc.vector.tensor_copy(out=hw_sb[:w_, :], in_=ps[:w_, :])
                    nc.sync.dma_start(out=hw_sh[0][n0:n0 + w_, :], in_=hw_sb[:w_, :])

            hw1_table()
            nc.gpsimd.collective_compute(
                "AllGather", ALU.bypass, replica_groups=rg,
                ins=[hw_sh[0][:]], outs=[hw_full[0][:]])

            def aggregate(hwf, hgdst, bias_ap, with_c0):
                ci = 0
                for b in range(NBLK):
                    ps = PS.tile([128, 128], dt.float32, tag="small")
                    nsteps = C[b] + (1 if with_c0 else 0)
                    step = 0
                    if with_c0:
                        negt = IO3.tile([1, 128], dt.bfloat16, tag="negt")
                        nc.sync.dma_start(out=negt[:],
                                          in_=NEGS[:, b * 128:(b + 1) * 128])
                        nc.tensor.matmul(out=ps[:], lhsT=c0m[:], rhs=negt[:],
                                         start=True, stop=(nsteps == 1))
                        step += 1
                    for c in range(C[b]):
                        g_t = IO3.tile([128, 128], dt.bfloat16, tag="g", bufs=10)
                        nc.gpsimd.indirect_dma_start(
                            out=g_t[:], out_offset=None, in_=hwf[:],
                            in_offset=bass.IndirectOffsetOnAxis(
                                ap=idxt_sb[:, ci:ci + 1], axis=0))
                        sel_t = IO3.tile([128, 128], dt.bfloat16, tag="sel", bufs=10)
                        nc.sync.dma_start(out=sel_t[:], in_=SEL[ci])
                        nc.tensor.matmul(out=ps[:], lhsT=g_t[:], rhs=sel_t[:],
                                         start=(step == 0), stop=(step == nsteps - 1))
                        step += 1
                        ci += 1
                    nc.scalar.activation(out=hgdst[:, b * 128:(b + 1) * 128],
                                         in_=ps[:], func=AF.Relu, bias=bias_ap)

            aggregate(hw_full[0], hg1, bias_sb["b_c1"][:, 0:1], True)

            # ================= conv2 =================
            wc2_sb = PP.tile([128, 128], dt.bfloat16)
            nc.sync.dma_start(out=wc2_sb[:], in_=W["w_c2"][:])
            for nb in range(NBLK):
                w_ = 128 if nb < NBLK - 1 else 106
                n0 = nb * 128
                ps = PS.tile([128, 128], dt.float32, tag="small")
                nc.tensor.matmul(out=ps[:w_, :], lhsT=hg1[:, n0:n0 + w_],
                                 rhs=wc2_sb[:], start=True, stop=True)
                hw_sb = IO3.tile([128, 128], dt.bfloat16, tag="hwsb")
                nc.vector.tensor_copy(out=hw_sb[:w_, :], in_=ps[:w_, :])
                nc.sync.dma_start(out=hw_sh[1][n0:n0 + w_, :], in_=hw_sb[:w_, :])
            nc.gpsimd.collective_compute(
                "AllGather", ALU.bypass, replica_groups=rg,
                ins=[hw_sh[1][:]], outs=[hw_full[1][:]])
            aggregate(hw_full[1], hg2, bias_sb["b_c2"][:, 0:1], False)

            # ================= head =================
            wf1_sb = PP.tile([128, 64], dt.bfloat16)
            nc.sync.dma_start(out=wf1_sb[:], in_=W["w_f1"][:])
            wf2_sb = PP.tile([64, 1], dt.bfloat16)
            nc.sync.dma_start(out=wf2_sb[:], in_=W["w_f2"][:])
            for n in range(13):
                w_ = NCHUNKS[n]
                p1 = PS.tile([64, 512], dt.float32, tag="mm")
                nc.tensor.matmul(out=p1[:, :w_], lhsT=wf1_sb[:],
                                 rhs=hg2[:, NOFF[n]:NOFF[n] + w_],
                                 start=True, stop=True)
                f1sb = IO3.tile([64, 512], dt.bfloat16, tag="f1sb")
                nc.scalar.activation(out=f1sb[:, :w_], in_=p1[:, :w_],
                                     func=AF.Relu, bias=bias_sb["b_f1"][:, 0:1])
                p2 = PS.tile([1, 512], dt.float32, tag="mm")
                nc.tensor.matmul(out=p2[:, :w_], lhsT=wf2_sb[:], rhs=f1sb[:, :w_],
                                 start=True, stop=True)
                osb = IO3.tile([1, 512], dt.float32, tag="osb")
                nc.scalar.activation(out=osb[:, :w_], in_=p2[:, :w_],
                                     func=AF.Sigmoid, bias=bias_sb["b_f2"][:, 0:1])
                nc.sync.dma_start(out=out_d[:, NOFF[n]:NOFF[n] + w_],
                                  in_=osb[:, :w_])

    nc.compile()
    return nc


# ----------------------------------------------------------------------------
# entry point
# ----------------------------------------------------------------------------

def kernel(**inputs):
    import jax  # noqa: F401  (triggers axon boot via sitecustomize)
    from concourse import bass_utils

    x = np.asarray(inputs["x"], np.float32)
    edge_index = np.asarray(inputs["edge_index"])

    eb = edge_index.tobytes()
    key = ("k", edge_index.shape, hash(eb[:4096]), hash(eb[-4096:]))
    if key not in _CACHE:
        C, TOTC, SELs, IDXs, NEGSs = _prep_edges(edge_index)
        nc = _build(C, TOTC)
        _CACHE.clear()
        _CACHE[key] = (nc, C, TOTC, SELs, IDXs, NEGSs)
    nc, C, TOTC, SELs, IDXs, NEGSs = _CACHE[key]

    w = _prep_weights(inputs)
    in_maps = []
    for r in range(R):
        xr = np.zeros((FIN, NS), bf16)
        xr[:3299] = np.ascontiguousarray(x[r * NS:(r + 1) * NS].T).astype(bf16)
        m = {"xT": xr, "SEL": SELs[r], "IDXT": IDXs[r], "NEGS": NEGSs[r]}
        for nm in ["w_e1", "w_e2", "w_e3", "w_e4", "w_e5", "w_e6",
                   "wc1a", "wc1b", "w_c2", "w_f1", "w_f2",
                   "b_e1", "b_e2", "b_e3", "b_e4", "b_e5", "b_e6",
                   "b_c1", "b_c2", "b_f1", "b_f2", "ident"]:
            m[nm] = w[nm]
        in_maps.append(m)

    res = bass_utils.run_bass_kernel_spmd(nc, in_maps, core_ids=list(range(R)))
    out = np.concatenate([np.asarray(res.results[r]["out"][0], np.float32)
                          for r in range(R)])
    return out[:, None].astype(np.float32)
